# revision 29
# baseline (speedup 1.0000x reference)
"""Trainium2 Bass kernel for additive (Bahdanau-style) attention with coverage.

Reference computation (per batch b):
  wq[t,e]   = sum_d q[t,d] Wq[e,d]
  uhcv[e,s] = sum_d m[s,d] Wc[e,d] + Wcov[e]*cov[s] + bcov[e]
  align[t,s]= sum_e v[e] * tanh(wq[t,e] + uhcv[e,s])
  a         = softmax_s(align)
  c[t,d]    = sum_s a[t,s] m[s,d]
  attn[t,:] = [c,q] @ Wout^T + bout
Outputs: attn_h [T,B,D], a [T,B,S], cov+a [T,B,S].

Sharding: data-parallel over batch B=8 across the 8 NeuronCores; the small
weights are replicated (pre-transposed on host so no on-chip weight
transposes are needed).

Per-core layout: feature dim e on partitions (4 chunks of 128), s/t on the
free axis.  The wq[t,:] term is added per-partition with DVE tensor_scalar
in bf16 (4x mode), tanh runs on ACT over t-groups (large free dim amortizes
the per-instruction overhead; ACT is the bottleneck engine at ~1 elem/lane/
cycle for the inherent 16.8M tanh evals per core), and the v-dot uses PE
with the tanh tile as the stationary operand producing alignT[s,t] per
t-group (full 128-wide M; PE matmul output must start at a 32-aligned PSUM
partition, so per-t M=1 row scatter is not expressible).  Each group's
alignT gets exp'd in place (same ACT table set as tanh, no max-subtraction:
|align| < ~3 is safe in fp32), is PE-transposed back to [t,s] for the
softmax normalization, and its softmax/aT/cT flow overlaps the next group's
tanh work.  Group sizes (8,24,32) ramp up so the first tanh starts early.
All phase-1/phase-3 matmuls run in bf16 (fp32 PE matmul is multi-pass);
PSUM accumulation uses one group per 2KB bank (start clears the whole
zero region).  Measured ~152us per invocation across the 8 cores.
"""

import sys

for _p in ("/opt/trn_rl_repo",):
    if _p not in sys.path:
        sys.path.insert(0, _p)

import numpy as np
import ml_dtypes

T, B, S, D = 64, 8, 512, 512
NC = 8          # cores
CH = D // 128   # feature chunks = 4
TG = 32         # cov replication rows (max group size)
GROUPS = (8, 24, 24, 8)  # t-group sizes (sum = T); small tail group shrinks the serial epilogue

_compiled = None


def _build(repeats=1, loop_iters=0, bf16_args=True, abufs=2, w2bufs=2, psswap=True, ps3=False, probe=None, fast_start=True, split_attn=True, groups=None, fast_dma=True, warmup=8):
    import concourse.bacc as bacc
    import concourse.tile as tile
    from concourse import mybir
    from concourse.masks import make_identity

    F32 = mybir.dt.float32
    BF16 = mybir.dt.bfloat16
    Tanh = mybir.ActivationFunctionType.Tanh
    Exp = mybir.ActivationFunctionType.Exp

    nc = bacc.Bacc("TRN2", target_bir_lowering=False, debug=False, num_devices=NC)

    d_qT = nc.dram_tensor("qT", [D, T], BF16, kind="ExternalInput")
    d_m = nc.dram_tensor("m", [S, D], F32, kind="ExternalInput")
    d_mT = nc.dram_tensor("mT", [D, S], BF16, kind="ExternalInput")
    d_WqT = nc.dram_tensor("WqT", [D, D], BF16, kind="ExternalInput")
    d_WcT = nc.dram_tensor("WcT", [D, D], BF16, kind="ExternalInput")
    d_WoT = nc.dram_tensor("WoT", [2 * D, D], BF16, kind="ExternalInput")
    d_vp = nc.dram_tensor("vp", [128, CH], BF16, kind="ExternalInput")
    d_wcb = nc.dram_tensor("wcb", [2, D], BF16, kind="ExternalInput")
    d_cvo = nc.dram_tensor("cvo", [2, S], BF16, kind="ExternalInput")
    d_cov16 = nc.dram_tensor("cov16", [TG, S], F32, kind="ExternalInput")
    d_bout = nc.dram_tensor("bout", [1, D], F32, kind="ExternalInput")

    d_attn = nc.dram_tensor("attn", [T, D], F32, kind="ExternalOutput")
    d_alig = nc.dram_tensor("alig", [T, S], F32, kind="ExternalOutput")
    d_cov = nc.dram_tensor("cov", [T, S], F32, kind="ExternalOutput")

    with tile.TileContext(nc) as tc:
        from contextlib import ExitStack

        with ExitStack() as ctx:
            consts = ctx.enter_context(tc.tile_pool(name="consts", bufs=1))
            work = ctx.enter_context(tc.tile_pool(name="work", bufs=1))
            work2 = ctx.enter_context(tc.tile_pool(name="work2", bufs=w2bufs))
            argp = ctx.enter_context(tc.tile_pool(name="argp", bufs=abufs))
            tanhp = ctx.enter_context(tc.tile_pool(name="tanhp", bufs=abufs))
            # PSUM budget (8 banks): uh/wq 2, cT 1, alignT 1, sm 2, attn 1, aT 1
            # (psswap: two softmax banks let group g+1's transposes overlap
            #  group g's exp/reduce; alignT needs only one since its reader
            #  (exp) runs immediately after the group's last matmul)
            psUh = ctx.enter_context(tc.tile_pool(name="psUh", bufs=1 if ps3 else 2, space="PSUM"))
            psMisc = ctx.enter_context(tc.tile_pool(name="psMisc", bufs=1, space="PSUM"))
            psAlign = ctx.enter_context(tc.tile_pool(name="psAlign", bufs=1 if psswap else 2, space="PSUM"))
            psSm = ctx.enter_context(tc.tile_pool(name="psSm", bufs=3 if ps3 else (2 if psswap else 1), space="PSUM"))
            psAttn = ctx.enter_context(tc.tile_pool(name="psAttn", bufs=1, space="PSUM"))
            psAT = ctx.enter_context(tc.tile_pool(name="psAT", bufs=1, space="PSUM"))

            def body():
                _WcT_r = d_WcT.ap().rearrange("(c p) e -> p c e", p=128)
                _mT_r = d_mT.ap().rearrange("(c p) s -> p c s", p=128)
                _WqT_r = d_WqT.ap().rearrange("(c p) e -> p c e", p=128)
                t_WcT = consts.tile([128, CH, D], BF16, tag="WcT")
                t_mT = consts.tile([128, CH, S], BF16, tag="mT")
                t_qT = consts.tile([128, CH, T], BF16, tag="qT")
                t_WqT = consts.tile([128, CH, D], BF16, tag="WqT")
                t_wcb = consts.tile([2, D], BF16, tag="wcb")
                t_cvo = consts.tile([2, S], BF16, tag="cvo")
                t_vp = consts.tile([128, CH], BF16, tag="vp")
                t_cov16 = consts.tile([TG, S], F32, tag="cov16")
                t_m = consts.tile([128, CH, D], F32, tag="m")
                t_WoT = consts.tile([128, 2 * CH, D], BF16, tag="WoT")
                t_bout = consts.tile([1, D], F32, tag="bout")
                _big_dma = probe != "nodma"
                _w = S if _big_dma else 16
                _wq_t = T if _big_dma else 16
                if fast_dma:
                    # Everything on the sync/HWDGE queue in consumption order
                    # (one consolidated dma_start per tensor; HWDGE descriptor
                    # gen is ~625ns each on its own unit, keeping Pool free
                    # for make_identity).  PE program order is wq -> uh, so
                    # WqT/qT lead, then WcT/mT.
                    t_ident = consts.tile([128, 128], F32, tag="ident")
                    make_identity(nc, t_ident[:, :])
                    # Transfer order IS the critical path to the first tanh:
                    # uh(ec0,s-half0) needs WcT + mT[:,:,0:256]; the g0c0 args
                    # additionally need wq(ec0) = WqT[:,:,0:128] @ qT and the
                    # wcb/cvo bias term.  Splitting mT and WqT lets the first
                    # tanh start ~5us earlier than whole-tensor ordering.
                    nc.sync.dma_start(out=t_WcT[:, :, :], in_=_WcT_r[:, :, :])
                    nc.sync.dma_start(out=t_mT[:, :, 0:S // 2], in_=_mT_r[:, :, 0:S // 2])
                    nc.sync.dma_start(out=t_wcb[:, :], in_=d_wcb.ap()[:, :])
                    nc.sync.dma_start(out=t_cvo[:, :], in_=d_cvo.ap()[:, :])
                    nc.sync.dma_start(out=t_qT[:, :, :], in_=d_qT.ap().rearrange("(c p) t -> p c t", p=128)[:, :, :])
                    nc.sync.dma_start(out=t_WqT[:, :, 0:128], in_=_WqT_r[:, :, 0:128])
                    nc.sync.dma_start(out=t_mT[:, :, S // 2:], in_=_mT_r[:, :, S // 2:])
                    nc.sync.dma_start(out=t_WqT[:, :, 128:], in_=_WqT_r[:, :, 128:])
                    nc.sync.dma_start(out=t_vp[:, :], in_=d_vp.ap()[:, :])
                    nc.sync.dma_start(out=t_cov16[:, :], in_=d_cov16.ap()[:, :])
                    # m/WoT/bout are needed only ~25us in; same HWDGE queue
                    # AFTER the critical tensors so their 1MB transfers can't
                    # jump ahead of WcT/mT on the DMA rings (gpsimd SWDGE gens
                    # complete early and would reorder them).
                    nc.sync.dma_start(out=t_m[:, :, :], in_=d_m.ap().rearrange("(c p) d -> p c d", p=128)[:, :, :])
                    nc.sync.dma_start(out=t_WoT[:, :, :], in_=d_WoT.ap().rearrange("(c p) e -> p c e", p=128)[:, :, :])
                    nc.sync.dma_start(out=t_bout[:, :], in_=d_bout.ap()[:, :])
                else:
                    for kc in range(CH):
                        nc.gpsimd.dma_start(out=t_WcT[:, kc, 0:_w], in_=_WcT_r[:, kc, 0:_w])
                        nc.gpsimd.dma_start(out=t_mT[:, kc, 0:_w], in_=_mT_r[:, kc, 0:_w])
                    nc.sync.dma_start(out=t_qT[:, :, 0:_wq_t], in_=d_qT.ap().rearrange("(c p) t -> p c t", p=128)[:, :, 0:_wq_t])
                    nc.sync.dma_start(out=t_wcb[:, :], in_=d_wcb.ap()[:, :])
                    nc.sync.dma_start(out=t_cvo[:, :], in_=d_cvo.ap()[:, :])
                    nc.sync.dma_start(out=t_vp[:, :], in_=d_vp.ap()[:, :])
                    nc.sync.dma_start(out=t_WqT[:, :, 0:_w], in_=_WqT_r[:, :, 0:_w])
                    nc.sync.dma_start(out=t_cov16[:, :], in_=d_cov16.ap()[:, :])
                    nc.gpsimd.dma_start(out=t_m[:, :, 0:_w], in_=d_m.ap().rearrange("(c p) d -> p c d", p=128)[:, :, 0:_w])
                    nc.gpsimd.dma_start(out=t_WoT[:, :, 0:_w], in_=d_WoT.ap().rearrange("(c p) e -> p c e", p=128)[:, :, 0:_w])
                    nc.gpsimd.dma_start(out=t_bout[:, :], in_=d_bout.ap()[:, :])

                if not fast_dma:
                    t_ident = consts.tile([128, 128], F32, tag="ident")
                    make_identity(nc, t_ident[:, :])
                t_ones = consts.tile([1, T], F32, tag="ones")
                nc.vector.memset(t_ones[:, :], 1.0)
                t_ones128 = consts.tile([128, 1], F32, tag="ones128")
                nc.vector.memset(t_ones128[:, :], 1.0)

                if warmup:
                    # p-state ramp: keep PE continuously busy (~3us) before
                    # the first real matmul so wq/uh run at full clock.
                    ps_warm = psUh.tile([128, S], F32, tag="ps_uh")
                    for _wm in range(warmup):
                        nc.tensor.matmul(
                            ps_warm[:, 0:128], t_ident[:, :], t_ident[:, :],
                            start=True, stop=True, skip_group_check=True,
                        )

                # ---- wq[e,t] = sum_d WqT[d,e] qT[d,t] -----------------------
                # one accumulation group per PSUM bank: start only on the
                # globally first matmul into the bank, stop on the last (start
                # clears has_written for the whole 2KB zero region).
                # ec=0 first (with its own copy) so group 0 can start early;
                # uh ec=0 interleaves right after.
                ARGDT = BF16 if bf16_args else F32
                t_wq = work.tile([128, CH, T], F32, tag="wq")
                t_uhcv = work.tile([128, CH, S], ARGDT, tag="uhcv")

                def emit_wq(ec):
                    ps_wq = psUh.tile([128, T], F32, tag="ps_uh")
                    for kc in range(CH):
                        nc.tensor.matmul(
                            ps_wq[:, :],
                            t_WqT[:, kc, ec * 128:(ec + 1) * 128],
                            t_qT[:, kc, :],
                            start=(kc == 0),
                            stop=(kc == CH - 1),
                        )
                    nc.vector.tensor_copy(t_wq[:, ec, :], ps_wq[:, :])

                def emit_uh(ec, spans=((0, S),)):
                    ps_uh = psUh.tile([128, S], F32, tag="ps_uh")
                    split = len(spans) > 1
                    for si, (s0, s1) in enumerate(spans):
                        for kc in range(CH):
                            nc.tensor.matmul(
                                ps_uh[:, s0:s1],
                                t_WcT[:, kc, ec * 128:(ec + 1) * 128],
                                t_mT[:, kc, s0:s1],
                                start=(si == 0 and kc == 0),
                                stop=False,
                                skip_group_check=split,
                            )
                        nc.tensor.matmul(
                            ps_uh[:, s0:s1],
                            t_wcb[:, ec * 128:(ec + 1) * 128],
                            t_cvo[:, s0:s1],
                            start=False,
                            stop=(si == len(spans) - 1),
                            skip_group_check=split,
                        )
                        if split:
                            nc.vector.tensor_copy(t_uhcv[:, ec, s0:s1], ps_uh[:, s0:s1])
                    if not split:
                        nc.vector.tensor_copy(t_uhcv[:, ec, :], ps_uh[:, :])

                _uh_spans = ((0, S // 2), (S // 2, S)) if fast_start else ((0, S),)
                if fast_dma:
                    # uh(ec0) first, split by s-halves so the first half's
                    # matmuls+copy run as soon as mT half 0 lands; wq(ec0)
                    # interleaves on PE right after half 0's wcb term.
                    # Chunks 1..3 are emitted from inside group 0's chunk loop
                    # so their PSUM->SBUF copies (stuck behind late DMAs)
                    # don't block the g0c0 args on the in-order DVE queue.
                    emit_uh(0, _uh_spans)
                    emit_wq(0)
                else:
                    emit_wq(0)
                    emit_uh(0, _uh_spans)
                    for ec in range(1, CH):
                        emit_wq(ec)
                        emit_uh(ec)

                # attn q-side partial sums: emitted from the group loop once
                # WoT has landed (see below).  ps_attn matmuls bypass the
                # sim's group bookkeeping: the two 32-row halves close at
                # different times and the tracker is partition-offset-blind;
                # on HW only `start` (zero region) matters and exactly one
                # start is issued.
                ps_attn = psAttn.tile([T, D], F32, tag="ps_attn")
                t_qproj = work.tile([32, 2, D], F32, tag="qproj")

                def emit_attn_q():
                    for k2 in range(CH, 2 * CH):
                        nc.tensor.matmul(
                            ps_attn[:, :], t_qT[:, k2 - CH, :], t_WoT[:, k2, :],
                            start=(k2 == CH), stop=False, skip_group_check=True,
                        )
                    nc.tensor.matmul(
                        ps_attn[:, :], t_ones[0:1, :], t_bout[0:1, :],
                        start=False, stop=True, skip_group_check=True,
                    )
                    # q-side partials to SBUF (early, off the critical path)
                    # so the final stt reads only one PSUM operand; per-half
                    # at partition base 0 to match the rcpH scalar's base
                    nc.vector.tensor_copy(t_qproj[:, 0, :], ps_attn[0:32, :])
                    nc.vector.tensor_copy(t_qproj[:, 1, :], ps_attn[32:64, :])

                if not fast_dma:
                    emit_attn_q()

                # ---- main loop over t-groups --------------------------------
                # cT'[d,t] = sum_s m[s,d] exp(align)[s,t] is computed from the
                # UNNORMALIZED exp tile (t_expT, layout [s_p, c, t]) straight
                # off the ACT output — no a-transpose round trip.  The softmax
                # normalization is a per-t scalar, applied at the very end as
                # attn[t,:] = rcp[t]*cTproj[t,:] + qproj[t,:] (one DVE
                # scalar_tensor_tensor on per-partition rcp).
                ps_attnC = psAT.tile([T, D], F32, tag="ps_attnC")
                ps_cT = psMisc.tile([128, CH, T], F32, tag="ps_misc")
                t_cT = work.tile([128, CH, T], BF16, tag="cT")
                t_expH = None
                half_rcps = []
                _groups = tuple(groups) if groups is not None else GROUPS
                assert sum(_groups) == T and max(_groups) <= TG
                n_groups = len(_groups)
                g_off = [sum(_groups[:i]) for i in range(n_groups)]
                for g in range(n_groups):
                    gsz = _groups[g]
                    hb = 32 * (g_off[g] // 32)   # 32-row half this group is in
                    off = g_off[g] - hb          # column offset inside the half
                    ps_alB = psAlign.tile([128, CH * TG + 2], F32, tag="ps_alT")
                    ps_alT = ps_alB[:, 0:CH * TG].rearrange("p (c t) -> p c t", c=CH)
                    ps_sum = ps_alB[:, CH * TG:CH * TG + 1]
                    ps_sumH = ps_alB[:, CH * TG + 1:CH * TG + 2]
                    for c in range(CH):
                        if fast_dma and g == 0 and c >= 1:
                            emit_wq(c)
                            emit_uh(c)
                        t_arg = argp.tile([128, TG, S], ARGDT, tag="arg")
                        _ntl = 1 if probe == "nodve" else gsz
                        _halves = (
                            [(0, S // 2), (S // 2, S)]
                            if (fast_start and g == 0 and c == 0)
                            else [(0, S)]
                        )
                        for s0, s1 in _halves:
                            for tl in range(_ntl):
                                t_idx = g_off[g] + tl
                                nc.vector.tensor_scalar_add(
                                    t_arg[:, tl, s0:s1],
                                    t_uhcv[:, c, s0:s1],
                                    t_wq[:, c, t_idx:t_idx + 1],
                                )
                        t_tanh = tanhp.tile([128, TG, S], BF16, tag="tanh")
                        _asz = gsz // 2 if probe == "halfact" else gsz
                        for s0, s1 in _halves:
                            nc.scalar.activation(
                                t_tanh[:, 0:_asz, s0:s1], t_arg[:, 0:_asz, s0:s1], Tanh)
                        _clast = 0 if probe == "nope" else CH - 1
                        for tl in range(gsz):
                            for sb in range(CH):
                                if probe == "nope" and c > 0:
                                    continue
                                nc.tensor.matmul(
                                    ps_alT[:, sb, tl:tl + 1],
                                    t_tanh[:, tl, sb * 128:(sb + 1) * 128],
                                    t_vp[:, c:c + 1],
                                    start=(c == 0 and tl == 0 and sb == 0),
                                    stop=(c == _clast and tl == gsz - 1 and sb == CH - 1),
                                )

                    if fast_dma and g == 0:
                        emit_attn_q()

                    # per-group softmax, overlapping the next group.  exp for
                    # the whole 32-row half accumulates into one tile so the
                    # half-level softmax sums (for the late attn
                    # normalization) land at partition 0 (32-aligned).
                    if off == 0:
                        t_expH = work2.tile([128, CH, 32], F32, tag="expT")
                    osl = slice(off, off + gsz)
                    nc.scalar.activation(t_expH[:, :, osl], ps_alT[:, :, 0:gsz], Exp)
                    ps_al2 = psSm.tile([TG, CH, 128], F32, tag="ps_sm")
                    for sb in range(CH):
                        nc.tensor.transpose(
                            ps_al2[0:gsz, sb, :], t_expH[:, sb, osl], t_ident[:, :]
                        )
                    gsl = slice(g_off[g], g_off[g] + gsz)
                    # cT'[d, g-cols] = sum_s m[s,d] expT[s, g-cols] straight
                    # from the exp tile (unnormalized).  Emitted before the
                    # softmax reduce so PE gets going as soon as exp lands.
                    # The cT bank group opens/closes per 32-column half so the
                    # first half's output-projection matmuls can run while
                    # later groups are still in their tanh phase.
                    _h_start = off == 0
                    _h_end = off + gsz == 32
                    for dc in range(CH):
                        for sc in range(CH):
                            nc.tensor.matmul(
                                ps_cT[:, dc, gsl],
                                t_m[:, sc, dc * 128:(dc + 1) * 128],
                                t_expH[:, sc, osl],
                                start=(_h_start and dc == 0 and sc == 0),
                                stop=(_h_end and dc == CH - 1 and sc == CH - 1),
                            )
                    # group softmax sums via PE: rsum[t] = sum_s expT[s, t] as
                    # four K=128 matmuls against a ones column (contraction
                    # over the s partitions) — off the DVE critical chain.
                    for sb in range(CH):
                        nc.tensor.matmul(
                            ps_sum[0:gsz, :],
                            t_expH[:, sb, osl],
                            t_ones128[:, :],
                            start=(sb == 0),
                            stop=(sb == CH - 1),
                            skip_group_check=True,
                        )
                    t_rcp = work2.tile([TG, 1], F32, tag="rcp")
                    nc.vector.reciprocal(t_rcp[0:gsz, :], ps_sum[0:gsz, :])
                    t_a = work2.tile([TG, S], F32, tag="a")
                    nc.vector.tensor_scalar_mul(
                        t_a[0:gsz, :],
                        ps_al2[0:gsz, :, :].rearrange("t c p -> t (c p)"),
                        t_rcp[0:gsz, 0:1])
                    nc.sync.dma_start(out=d_alig.ap()[gsl, :], in_=t_a[0:gsz, :])
                    t_cn = work2.tile([TG, S], F32, tag="cn")
                    nc.vector.tensor_add(t_cn[0:gsz, :], t_a[0:gsz, :], t_cov16[0:gsz, :])
                    nc.sync.dma_start(out=d_cov.ap()[gsl, :], in_=t_cn[0:gsz, :])
                    if _h_end:
                        # half-level softmax sums + rcp (partition 0 base) for
                        # the deferred attn normalization
                        for sb in range(CH):
                            nc.tensor.matmul(
                                ps_sumH[0:32, :],
                                t_expH[:, sb, 0:32],
                                t_ones128[:, :],
                                start=(sb == 0),
                                stop=(sb == CH - 1),
                                skip_group_check=True,
                            )
                        t_rcpH = work2.tile([32, 1], F32, tag="rcpH")
                        nc.vector.reciprocal(t_rcpH[:, :], ps_sumH[0:32, :])
                        half_rcps.append(t_rcpH)
                    if _h_end and split_attn:
                        # this half's rows of attn: cT' copy + bf16 matmuls of
                        # the c-part into ps_attnC rows (32-aligned base
                        # partition), then one DVE stt applies the softmax
                        # normalization and adds the q-part + bias.
                        hsl = slice(hb, hb + 32)
                        nc.vector.tensor_copy(t_cT[:, :, hsl], ps_cT[:, :, hsl])
                        for k2 in range(CH):
                            nc.tensor.matmul(
                                ps_attnC[hsl, :], t_cT[:, k2, hsl], t_WoT[:, k2, :],
                                start=(hb == 0 and k2 == 0), stop=(hb != 0 and k2 == CH - 1),
                                skip_group_check=True,
                                tile_position=(0, hb) if hb else None,
                            )
                        t_attn = work2.tile([32, D], F32, tag="attn_h")
                        nc.vector.scalar_tensor_tensor(
                            t_attn[:, :], ps_attnC[hsl, :], t_rcpH[:, 0:1],
                            t_qproj[:, hb // 32, :],
                            mybir.AluOpType.mult, mybir.AluOpType.add,
                        )
                        nc.sync.dma_start(out=d_attn.ap()[hsl, :], in_=t_attn[:, :])



                if not split_attn:
                    nc.vector.tensor_copy(t_cT[:, :, :], ps_cT[:, :, :])
                    for k2 in range(CH):
                        nc.tensor.matmul(
                            ps_attnC[:, :], t_cT[:, k2, :], t_WoT[:, k2, :],
                            start=(k2 == 0), stop=(k2 == CH - 1),
                            skip_group_check=True,
                        )
                    t_attn_f = work.tile([T, D], F32, tag="attn_f")
                    for hi, hb2 in enumerate((0, 32)):
                        hsl = slice(hb2, hb2 + 32)
                        nc.vector.scalar_tensor_tensor(
                            t_attn_f[hsl, :], ps_attnC[hsl, :],
                            half_rcps[hi][:, 0:1],
                            t_qproj[:, hi, :],
                            mybir.AluOpType.mult, mybir.AluOpType.add,
                        )
                    nc.sync.dma_start(out=d_attn.ap()[:, :], in_=t_attn_f[:, :])

            if loop_iters:
                with tc.For_i(0, loop_iters, 1,
                              hint_engines=(mybir.EngineType.PE,
                                            mybir.EngineType.DVE,
                                            mybir.EngineType.Pool,
                                            mybir.EngineType.SP)):
                    body()
            else:
                for _rep in range(repeats):
                    body()

    nc.compile()
    return nc


def _get_compiled():
    global _compiled
    if _compiled is None:
        _compiled = _build()
    return _compiled


def make_in_maps(input, memory_bank, cov_vec, Wq, Wc, Wcov, bcov, v, Wout, bout):
    f32 = np.float32
    input = np.asarray(input, f32)
    memory_bank = np.asarray(memory_bank, f32)
    cov_vec = np.asarray(cov_vec, f32)
    bf16 = ml_dtypes.bfloat16
    WqT = np.ascontiguousarray(np.asarray(Wq, f32).T.astype(bf16))
    WcT = np.ascontiguousarray(np.asarray(Wc, f32).T.astype(bf16))
    WoT = np.ascontiguousarray(np.asarray(Wout, f32).T.astype(ml_dtypes.bfloat16))
    vp = np.ascontiguousarray(
        np.asarray(v, f32).reshape(CH, 128).T.astype(ml_dtypes.bfloat16)
    )
    wcb = np.ascontiguousarray(
        np.stack([np.asarray(Wcov, f32)[:, 0], np.asarray(bcov, f32)]).astype(bf16)
    )
    bout_row = np.ascontiguousarray(np.asarray(bout, f32)[None, :])
    ones_row = np.ones((S,), f32)

    in_maps = []
    for b in range(NC):
        qT = np.ascontiguousarray(input[:, b, :].T.astype(bf16))
        m_b = np.ascontiguousarray(memory_bank[:, b, :])
        mT_b = np.ascontiguousarray(m_b.T.astype(bf16))
        cvo = np.ascontiguousarray(np.stack([cov_vec[b], ones_row]).astype(bf16))
        cov16 = np.ascontiguousarray(np.broadcast_to(cov_vec[b], (TG, S)))
        in_maps.append({
            "qT": qT, "m": m_b, "mT": mT_b,
            "WqT": WqT, "WcT": WcT, "WoT": WoT,
            "vp": vp, "wcb": wcb, "cvo": cvo,
            "cov16": cov16, "bout": bout_row,
        })
    return in_maps


def gather_outputs(results):
    attn_h = np.stack([results[b]["attn"] for b in range(NC)], axis=1)
    align_tb = np.stack([results[b]["alig"] for b in range(NC)], axis=1)
    cov_new = np.stack([results[b]["cov"] for b in range(NC)], axis=1)
    return attn_h, align_tb, cov_new


def kernel(**inputs):
    from concourse.bass_utils import run_bass_kernel_spmd

    nc = _get_compiled()
    in_maps = make_in_maps(**inputs)
    res = run_bass_kernel_spmd(nc, in_maps, core_ids=list(range(NC)))
    return gather_outputs(res.results)

